# revision 2
# baseline (speedup 1.0000x reference)
"""Trainium2 Bass kernel for nn_CrossOutLayer_2 (dense pairwise MLP).

o[b,n,m] = sum_e W2[e] * gelu(u[b,n,e] + v[b,m,e]) + b2
  u = x0 @ W1[:D] + x @ W1[D:2D] + b1,  v = y @ W1[2D:]

Instead of evaluating gelu on the full [b,n,m,e] grid (8M elems/core,
~55us on ACT), expand gelu in a separable basis:

  gelu(s) ~= al0 + al1*s + al2*s^2 + sum_k a_k cos(k*w0*s),
  k in {1,2,3,4,6}, w0 = pi/8, max err 2.6e-3 on s in [-6.6, 6.6].

Each cos(k*w0*(u+v)) = ck(u)ck(v) - sk(u)sk(v) factors into rank-2
products, so the pairwise reduction over e becomes 13 small PE matmuls
[128e x 128n] @ [128e x 512m] accumulated in PSUM. ACT only evaluates
sin/cos/square on the small u [128,128] and v [128,512] grids (all args
within the Sin table's valid domain |x|<=3.4); higher harmonics come
from exact Square frequency-doubling (cos2t = 2cos^2 t - 1, offsets
absorbed into row/col/const rank terms) and DVE products
(sin2t = 2 sint cost, scale factors folded into per-term coefficient
columns gamma*w2 uploaded from host).

Sharded over (b, n1): each core owns 128 rows of (b*512+n1), full m.
Output per core is o[n, m] directly (no transpose needed on host).
"""

import sys

sys.path.insert(0, "/opt/trn_rl_repo")

import numpy as np

B, N1, N2, D = 2, 512, 512, 128
NCORES = 8
ROWS = B * N1 // NCORES  # 128 (b,n1)-rows per core

# --- gelu cos-basis fit, R=8, ks=(1,2,3,4,6), maxerr 2.55e-3 ---
R_FIT = 8.0
W0 = float(np.pi / R_FIT)
AL0 = -0.01540275
AL1 = 0.5
AL2 = 0.09443302
A1 = 0.71274793
A2 = -0.59297207
A3 = 0.02119623
A4 = -0.10519945
A6 = -0.01809907
SQ_AL2 = float(np.sqrt(AL2))
BU = float(AL1 / (2 * SQ_AL2))  # Square-affine linear-term shift
CCONST = AL0 + A4 + A6 - 2 * BU * BU
SQ2 = float(np.sqrt(2.0))

NC_COLS = 21

_cache = {}


def _build(repeat=1):
    key = ("nc", repeat)
    if key in _cache:
        return _cache[key]
    import concourse.bacc as bacc
    import concourse.mybir as mybir
    import concourse.tile as tile

    f32 = mybir.dt.float32
    f16 = mybir.dt.float16
    Sin = mybir.ActivationFunctionType.Sin
    Square = mybir.ActivationFunctionType.Square
    mult = mybir.AluOpType.mult
    add = mybir.AluOpType.add

    nc = bacc.Bacc("TRN2", target_bir_lowering=False, debug=False)
    x0T = nc.dram_tensor("x0T", [D, ROWS], f32, kind="ExternalInput")
    xT = nc.dram_tensor("xT", [D, ROWS], f32, kind="ExternalInput")
    yT = nc.dram_tensor("yT", [D, N2], f32, kind="ExternalInput")
    Wa = nc.dram_tensor("Wa", [D, D], f32, kind="ExternalInput")
    Wb = nc.dram_tensor("Wb", [D, D], f32, kind="ExternalInput")
    Wc = nc.dram_tensor("Wc", [D, D], f32, kind="ExternalInput")
    cpk = nc.dram_tensor("cpk", [D, NC_COLS], f32, kind="ExternalInput")
    outT = nc.dram_tensor("outT", [ROWS, N2], f32, kind="ExternalOutput")

    with tile.TileContext(nc) as tc:
        with (
            tc.tile_pool(name="const", bufs=1) as cpool,
            tc.tile_pool(name="psum", bufs=1, space="PSUM") as pspool,
        ):

            def body():
                x0T_sb = cpool.tile([D, ROWS], f32, name="x0T_sb", tag="x0T_sb")
                nc.sync.dma_start(x0T_sb[:], x0T[:])
                xT_sb = cpool.tile([D, ROWS], f32, name="xT_sb", tag="xT_sb")
                nc.sync.dma_start(xT_sb[:], xT[:])
                Wa_sb = cpool.tile([D, D], f32, name="Wa_sb", tag="Wa_sb")
                nc.sync.dma_start(Wa_sb[:], Wa[:])
                Wb_sb = cpool.tile([D, D], f32, name="Wb_sb", tag="Wb_sb")
                nc.sync.dma_start(Wb_sb[:], Wb[:])
                Wc_sb = cpool.tile([D, D], f32, name="Wc_sb", tag="Wc_sb")
                nc.sync.dma_start(Wc_sb[:], Wc[:])
                cp_sb = cpool.tile([D, NC_COLS], f32, name="cp_sb", tag="cp_sb")
                nc.sync.dma_start(cp_sb[:], cpk[:])
                yT_sb = cpool.tile([D, N2], f32, name="yT_sb", tag="yT_sb")
                nc.sync.dma_start(yT_sb[:, : N2 // 2], yT[:, : N2 // 2])
                nc.sync.dma_start(yT_sb[:, N2 // 2 :], yT[:, N2 // 2 :])

                def cp(i):
                    return cp_sb[:, i : i + 1]

                # --- PE pre-GEMMs: hx[e,n], hy[e,m] (PSUM f32) ---
                hx_ps = pspool.tile([D, ROWS], f32, name="hx_ps", tag="hx")
                nc.tensor.matmul(hx_ps[:], Wa_sb[:], x0T_sb[:], start=True, stop=False)
                nc.tensor.matmul(hx_ps[:], Wb_sb[:], xT_sb[:], start=False, stop=True)
                hy_ps = pspool.tile([D, N2], f32, name="hy_ps", tag="hy")
                nc.tensor.matmul(hy_ps[:], Wc_sb[:], yT_sb[:], start=True, stop=True)

                def sb(name, w, dt):
                    return cpool.tile([D, w], dt, name=name, tag=name)

                # --- ACT u-side: sin/cos of w0*u, 2w0*u (b1 folded in bias) ---
                s1u = sb("s1u", ROWS, f32)
                nc.scalar.activation(s1u[:], hx_ps[:], Sin, bias=cp(0), scale=W0)
                c1u = sb("c1u", ROWS, f32)
                nc.scalar.activation(c1u[:], hx_ps[:], Sin, bias=cp(1), scale=W0)
                s2u = sb("s2u", ROWS, f32)
                nc.scalar.activation(s2u[:], hx_ps[:], Sin, bias=cp(2), scale=2 * W0)
                c2u = sb("c2u", ROWS, f32)
                nc.scalar.activation(c2u[:], hx_ps[:], Sin, bias=cp(3), scale=2 * W0)

                # --- ACT v-side: sin/cos of k*w0*v, k=1,2,3 (f16) ---
                s1v = sb("s1v", N2, f16)
                nc.scalar.activation(s1v[:], hy_ps[:], Sin, bias=0.0, scale=W0)
                c1v = sb("c1v", N2, f16)
                nc.scalar.activation(c1v[:], hy_ps[:], Sin, bias=cp(5), scale=W0)
                s2v = sb("s2v", N2, f16)
                nc.scalar.activation(s2v[:], hy_ps[:], Sin, bias=0.0, scale=2 * W0)
                c2v = sb("c2v", N2, f16)
                nc.scalar.activation(c2v[:], hy_ps[:], Sin, bias=cp(5), scale=2 * W0)
                s3v = sb("s3v", N2, f16)
                nc.scalar.activation(s3v[:], hy_ps[:], Sin, bias=0.0, scale=3 * W0)
                c3v = sb("c3v", N2, f16)
                nc.scalar.activation(c3v[:], hy_ps[:], Sin, bias=cp(5), scale=3 * W0)

                # --- DVE u-side products (f32): k=3 factors, stored sines ---
                p1 = sb("p1", ROWS, f32)
                nc.vector.tensor_mul(p1[:], c1u[:], c2u[:])
                p2 = sb("p2", ROWS, f32)
                nc.vector.tensor_mul(p2[:], s1u[:], s2u[:])
                c3u = sb("c3u", ROWS, f32)
                nc.vector.tensor_sub(c3u[:], p1[:], p2[:])
                p3 = sb("p3", ROWS, f32)
                nc.vector.tensor_mul(p3[:], s1u[:], c2u[:])
                p4 = sb("p4", ROWS, f32)
                nc.vector.tensor_mul(p4[:], c1u[:], s2u[:])
                s3u = sb("s3u", ROWS, f32)
                nc.vector.tensor_add(s3u[:], p3[:], p4[:])
                s4u = sb("s4u", ROWS, f32)
                nc.vector.tensor_mul(s4u[:], s2u[:], c2u[:])
                s6u = sb("s6u", ROWS, f32)
                nc.vector.tensor_mul(s6u[:], s3u[:], c3u[:])

                # --- ACT squares: q = 2*cos^2 (true cos2t = q-1) + quadratics
                q4u = sb("q4u", ROWS, f32)
                nc.scalar.activation(q4u[:], c2u[:], Square, bias=0.0, scale=SQ2)
                q6u = sb("q6u", ROWS, f32)
                nc.scalar.activation(q6u[:], c3u[:], Square, bias=0.0, scale=SQ2)
                q4v = sb("q4v", N2, f16)
                nc.scalar.activation(q4v[:], c2v[:], Square, bias=0.0, scale=SQ2)
                q6v = sb("q6v", N2, f16)
                nc.scalar.activation(q6v[:], c3v[:], Square, bias=0.0, scale=SQ2)
                pu = sb("pu", ROWS, f32)
                nc.scalar.activation(pu[:], hx_ps[:], Square, bias=cp(4), scale=SQ_AL2)
                qv = sb("qv", N2, f32)
                nc.scalar.activation(qv[:], hy_ps[:], Square, bias=cp(6), scale=SQ_AL2)

                # --- DVE v-side stored sines (f16) ---
                s4v = sb("s4v", N2, f16)
                nc.vector.tensor_mul(s4v[:], s2v[:], c2v[:])
                s6v = sb("s6v", N2, f16)
                nc.vector.tensor_mul(s6v[:], s3v[:], c3v[:])

                # --- DVE u-side folds -> f16 stationaries (gamma*w2 columns) ---
                def fold(name, src, col):
                    t = sb(name, ROWS, f16)
                    nc.vector.tensor_scalar(
                        out=t[:], in0=src[:], scalar1=cp(col), scalar2=None, op0=mult
                    )
                    return t

                st_c1 = fold("st_c1", c1u, 7)
                st_s1 = fold("st_s1", s1u, 8)
                st_c2 = fold("st_c2", c2u, 9)
                st_s2 = fold("st_s2", s2u, 10)
                st_c3 = fold("st_c3", c3u, 11)
                st_s3 = fold("st_s3", s3u, 12)
                st_q4 = fold("st_q4", q4u, 13)
                st_s4 = fold("st_s4", s4u, 14)
                st_q6 = fold("st_q6", q6u, 15)
                st_s6 = fold("st_s6", s6u, 16)
                st_uv = sb("st_uv", ROWS, f16)
                nc.vector.tensor_scalar(
                    out=st_uv[:], in0=hx_ps[:], scalar1=cp(17), scalar2=cp(18),
                    op0=mult, op1=add,
                )

                # --- rowU = pu - A4*q4u - A6*q6u, fold w2 -> f16 ---
                r1 = sb("r1", ROWS, f32)
                nc.vector.tensor_scalar(
                    out=r1[:], in0=q4u[:], scalar1=-A4, scalar2=None, op0=mult
                )
                ra = sb("ra", ROWS, f32)
                nc.vector.tensor_add(ra[:], pu[:], r1[:])
                r2 = sb("r2", ROWS, f32)
                nc.vector.tensor_scalar(
                    out=r2[:], in0=q6u[:], scalar1=-A6, scalar2=None, op0=mult
                )
                rb = sb("rb", ROWS, f32)
                nc.vector.tensor_add(rb[:], ra[:], r2[:])
                st_row = fold("st_row", rb, 19)

                # --- colV = w2*(qv - A4*q4v - A6*q6v) + kappa -> f16 ---
                h1 = sb("h1", N2, f16)
                nc.vector.tensor_scalar(
                    out=h1[:], in0=q4v[:], scalar1=-A4, scalar2=None, op0=mult
                )
                ca = sb("ca", N2, f32)
                nc.vector.tensor_add(ca[:], qv[:], h1[:])
                h2 = sb("h2", N2, f16)
                nc.vector.tensor_scalar(
                    out=h2[:], in0=q6v[:], scalar1=-A6, scalar2=None, op0=mult
                )
                cb = sb("cb", N2, f32)
                nc.vector.tensor_add(cb[:], ca[:], h2[:])
                colv = sb("colv", N2, f16)
                nc.vector.tensor_scalar(
                    out=colv[:], in0=cb[:], scalar1=cp(19), scalar2=cp(20),
                    op0=mult, op1=add,
                )

                v16 = sb("v16", N2, f16)
                nc.vector.tensor_copy(v16[:], hy_ps[:])
                ones_n = sb("ones_n", ROWS, f16)
                nc.vector.memset(ones_n[:], 1.0)
                ones_m = sb("ones_m", N2, f16)
                nc.vector.memset(ones_m[:], 1.0)

                # --- PE: 13 rank-term matmuls accumulate out[n,m] ---
                out_ps = pspool.tile([ROWS, N2], f32, name="out_ps", tag="out")
                terms = [
                    (st_c1, c1v), (st_s1, s1v), (st_c2, c2v), (st_s2, s2v),
                    (st_uv, v16), (st_c3, c3v), (st_s3, s3v), (st_q4, q4v),
                    (st_s4, s4v), (st_q6, q6v), (st_s6, s6v),
                    (st_row, ones_m), (ones_n, colv),
                ]
                for i, (stt, mov) in enumerate(terms):
                    nc.tensor.matmul(
                        out_ps[:], stt[:], mov[:],
                        start=(i == 0), stop=(i == len(terms) - 1),
                    )

                o_sb = cpool.tile([ROWS, N2], f32, name="o_sb", tag="o_sb")
                nc.vector.tensor_copy(o_sb[:], out_ps[:])
                nc.sync.dma_start(outT[:, : N2 // 2], o_sb[:, : N2 // 2])
                nc.sync.dma_start(outT[:, N2 // 2 :], o_sb[:, N2 // 2 :])

            if repeat == 1:
                body()
            else:
                with tc.For_i(0, repeat, 1, hint_engines=(mybir.EngineType.PE,)):
                    body()

    nc.compile()
    _cache[key] = nc
    return nc


def _prep_in_maps(x0, x, y, W1, b1, W2, b2):
    x0 = np.asarray(x0, np.float32)
    x = np.asarray(x, np.float32)
    y = np.asarray(y, np.float32)
    W1 = np.asarray(W1, np.float32)
    b1 = np.asarray(b1, np.float32)
    W2 = np.asarray(W2, np.float32)
    b2 = np.asarray(b2, np.float32)

    Wa = np.ascontiguousarray(W1[:D])
    Wb = np.ascontiguousarray(W1[D : 2 * D])
    Wc = np.ascontiguousarray(W1[2 * D :])
    w2 = W2[:, 0]

    cpk = np.zeros((D, NC_COLS), np.float32)
    hp = np.pi / 2
    cpk[:, 0] = W0 * b1
    cpk[:, 1] = W0 * b1 + hp
    cpk[:, 2] = 2 * W0 * b1
    cpk[:, 3] = 2 * W0 * b1 + hp
    cpk[:, 4] = SQ_AL2 * b1 + BU
    cpk[:, 5] = hp
    cpk[:, 6] = BU
    cpk[:, 7] = A1 * w2
    cpk[:, 8] = -A1 * w2
    cpk[:, 9] = A2 * w2
    cpk[:, 10] = -A2 * w2
    cpk[:, 11] = A3 * w2
    cpk[:, 12] = -A3 * w2
    cpk[:, 13] = A4 * w2
    cpk[:, 14] = -4 * A4 * w2
    cpk[:, 15] = A6 * w2
    cpk[:, 16] = -4 * A6 * w2
    cpk[:, 17] = 2 * AL2 * w2
    cpk[:, 18] = 2 * AL2 * w2 * b1
    cpk[:, 19] = w2
    cpk[:, 20] = w2 * CCONST + b2[0] / D
    cpk = np.ascontiguousarray(cpk)

    in_maps = []
    for c in range(NCORES):
        b = c // (N1 // ROWS)
        n0 = (c % (N1 // ROWS)) * ROWS
        in_maps.append(
            {
                "x0T": np.ascontiguousarray(x0[b, n0 : n0 + ROWS].T),
                "xT": np.ascontiguousarray(x[b, n0 : n0 + ROWS].T),
                "yT": np.ascontiguousarray(y[b].T),
                "Wa": Wa,
                "Wb": Wb,
                "Wc": Wc,
                "cpk": cpk,
            }
        )
    return in_maps


def kernel(x0, x, y, W1, b1, W2, b2):
    from concourse.bass_utils import run_bass_kernel_spmd

    nc = _build()
    in_maps = _prep_in_maps(x0, x, y, W1, b1, W2, b2)
    res = run_bass_kernel_spmd(nc, in_maps, list(range(NCORES)))
    kernel.last_result = res

    out = np.empty((B, N1, N2), np.float32)
    for c in range(NCORES):
        o = res.results[c]["outT"]  # [n, m]
        b = c // (N1 // ROWS)
        n0 = (c % (N1 // ROWS)) * ROWS
        out[b, n0 : n0 + ROWS] = o
    return out


kernel.last_result = None
